# revision 56
# baseline (speedup 1.0000x reference)
"""Trainium2 Bass kernel for nn_BottleneckFFN.

Computes y = LayerNorm(GELU(x @ W1.T + b1) @ W2.T + b2) * gamma + beta
for x of shape (128, 2048, 256), W1 (8, 256), W2 (8, 8), LN over the
trailing 8 channels.  Pure data parallel over 8 NeuronCores: the
128*2048 = 262144 token rows are split into 8 shards of 32768 tokens;
the tiny weights are replicated.

Per-core dataflow (per round of 2048 tokens), software-pipelined with a
1-round skew (loads/transposes for round r+1 are emitted before round
r's matmul stages):
  1. SWDGE cast-DMA (nc.gpsimd.dma_start with f32 in / bf16 out): the
     SDMA engines downcast inline, so 2 MB of HBM reads land directly
     as a 1 MB token-major bf16 tile ([128 part, 16 tiles, 256]).
     This deletes the old ACT(3.5us)+GpSimd(1.9us) per-round cast
     stage entirely -- the v1 trace showed those casts (plus the DVE
     transpose) starving the PE into HAM-cold oscillation (45%
     throttle time) and stretching the post-DMA drain to ~50us.
     bf16 matmuls keep full PE speed without float32r's
     psum-partition-base-0 restriction.  Measured rel err 3.5e-3
     (gate 2e-2).  Descriptor generation (~1us/round) runs on the
     GpSimd engine, which has nothing else to do.
  2. Two DVE 32x32 block transposes to feature-major per 32-partition
     group, on the bf16 tile BITCAST TO i32 so each transposed element
     is a packed pair of adjacent features: DVE transpose is
     element-rate-limited (~1.05 ns/col regardless of width), so the
     i32 packing halves DVE transpose time.  mm1 then reads even/odd
     features as stride-2 bf16 APs (per-column partition-parallel
     fetch makes PE cost stride-independent), with W1 reordered
     host-side to match (K-step k = 2*db32 + e contracts
     d = 64*db32 + 2a + e at partition 32P+a).
  3. mm1: 8 d-blocks x 4 concurrent diagonal K=32 bf16 matmuls
     (tile_position (32P, 32P)) accumulate x @ W1.T into ONE psum bank
     as [128, 512]: token group P's channels land at partitions
     32P..32P+32 (same-bank different-partition writes are safe).
     pp bufs=3 (and pp2 bufs=3; 6 of 8 banks) buffers the bank so
     mm1(r+1..r+2) overlap GELU(r).
  4. Exact GELU over all 128 lanes, b1 fused as per-partition bias,
     bf16 output feeding mm2 directly.  With the cast gone, GELU
     (~0.6us) is ACT's only steady-state work.
  5. mm2: 4 concurrent diagonal K=8 bf16 matmuls with a 32-col
     stationary whose col 8 is mean(W2 rows), so the per-token LN mean
     falls out of the matmul; fresh double-buffered psum bank.
  6. One DVE block-transpose back to token-major; centered = h2 - mu,
     Square, grouped reduce, all on DVE (~4.3us/round total DVE,
     under the 5.2us DMA round).
  7. Finalize per batch (rounds 0-8 / 8-14 / 14-16; the last batch's
     finalize is EMITTED after round 15's transpose+stats, so nothing
     queues ahead of the tail DVE chain):
     rstd = rsqrt(ssq/8 + eps) via magic-constant seed + one DVE
     Newton step -- table-free, so the ACT Gelu table is NEVER
     swapped (the old ACT Sqrt cost 2 x ~2.7us Gelu<->Sqrt table
     loads per finalize, parked between two GELUs that mm2 was
     waiting on).  The whole batch is then scaled in ONE wide DVE
     tensor_tensor (cent * rstd-broadcast over nr*128 cols) and
     stored with ONE multi-round HWDGE dma_start through a p-outer
     view of y (per partition the batch is a (round, f) 2D walk), so
     per-round scale/store op overhead and the DVE tail backlog both
     collapse; stores go out on the idle SP engine so nothing shares
     the GpSimd SWDGE descgen or the ACT GELU stream.

Key scheduling facts learned from traces (see git-less history in
kernel_v*.py):
  - The PE queue is in-order: mm2(r) emitted right after mm1(r) made
    the PE idle ~1.2-1.5us/round waiting on GELU(r).  Emitting mm2 one
    round late (after mm1(r+1)) removed the bubble: 126.8 -> 113.7us.
  - Round 0 is loaded as 4 independent sub-tiles with per-sub
    transposes and j-split mm1 so first matmuls issue at ~12.5us
    instead of ~15.5 (shorter fill, earlier HAM warm-up).
  - Steady state is at the HBM read roofline (~4.9-5.2us/round,
    ~430 GB/s effective); DVE is the closest compute engine
    (~4.6us/round).  Moving stats/scales to GpSimd (2-input TT) or
    folding the LN-mean into a centered mm2 stationary with persistent
    yt tiles both REGRESSED >13us on HW (GpSimd elementwise is far
    slower than its 2x-of-DVE billing; the persistent-yt variant
    re-cooled the PE) -- both were reverted.

v1 (HWDGE f32 loads + on-engine casts) measured 136.8us: loads ran at
~425 GB/s and finished by t=88us, but the cast+transpose+stats chain
could only retire rounds at ~6-10us in the drain, and the HAM-cold PE
(545ns/matmul = 1.2 GHz) stretched mm1.  Removing the cast (v2,
126.8us), the table-free finalize (v3), the mm2 bubble fix (v4,
113.7us), the round-0 sub-split (v5, 112.9us) and the merged
batch-wide scale+store (v8, 112.4us) landed at ~6.5us preamble +
~6us fill + ~80us of DMA-roofline rounds + ~13us tail+barrier.
Beware ~13% run-to-run drift on this part (HAM clock-gate phase +
cross-core HBM contention): identical binaries measured 112.9 and
128.7 in one session.  PE "heater" tricks were tried twice and
reverted: a 3.7us burst of K=32 matmuls at t=7.7-11.5 (memset-fed, so
it really did run in the fill window) did NOT trip HAM's un-throttle
-- the first K=8/8 event still came at t=33us -- so the activity
monitor evidently discounts low-array-utilization matmuls (1-4 of 16
sub-arrays; even the real diagonal mm1 only lights 4).  The warm/cold
lottery cannot be steered from the instruction stream; don't burn PE
time trying.
"""

import os
import sys

import numpy as np

if not any(os.path.isdir(os.path.join(p, "concourse")) for p in sys.path if p):
    for _cand in ("/opt/trn_rl_repo", "/root/.axon_site/_ro/trn_rl_repo"):
        if os.path.isdir(os.path.join(_cand, "concourse")):
            sys.path.insert(0, _cand)
            break

N_CORES = 8
DIM, OUT = 256, 8
B, T = 128, 2048
TOK_TOTAL = B * T
TOK_CORE = TOK_TOTAL // N_CORES  # 32768
R_TOK = 2048                     # tokens per round
N_R = TOK_CORE // R_TOK          # 16 rounds
J = R_TOK // 128                 # 16 [128, 256] tiles per round
JH = J // 2                      # 8 tiles per half-round
NDB = DIM // 32                  # 8 d-blocks of 32
EPS = 1e-5

_BUILD_CACHE = {}

# CoreSim doesn't implement Gelu; sim_test.py swaps in Tanh (and mirrors
# it in its numpy reference) to validate dataflow/layout off-hardware.
SIM_ACT_OVERRIDE = [None]


def build_kernel(use_b2c=False, use_gamma=False, use_beta=False,
                 repeat=1, variant="full"):
    """Build the per-core Bass program. Returns the compiled Bacc object."""
    key = (use_b2c, use_gamma, use_beta, repeat, variant,
           str(SIM_ACT_OVERRIDE[0]))
    if key in _BUILD_CACHE:
        return _BUILD_CACHE[key]

    import concourse.bacc as bacc
    import concourse.mybir as mybir
    from concourse.tile import TileContext

    f32 = mybir.dt.float32
    bf16 = mybir.dt.bfloat16
    AF = mybir.ActivationFunctionType
    ALU = mybir.AluOpType

    nc = bacc.Bacc("TRN2")
    x_d = nc.dram_tensor("x", [TOK_CORE, DIM], f32, kind="ExternalInput")
    # f32 consts: col 0 b1 (replicated per 32-group), 8:16 b2-mean(b2),
    # 16:24 gamma, 24:32 beta
    wp_d = nc.dram_tensor("wpack", [128, 32], f32, kind="ExternalInput")
    # bf16 consts: cols 0:256 w1t blocks, 256:288 w2t9 (replicated per
    # 32-group)
    wb_d = nc.dram_tensor("wpackb", [128, 288], bf16, kind="ExternalInput")
    y_d = nc.dram_tensor("y", [TOK_CORE, OUT], f32, kind="ExternalOutput")

    # token t = r*2048 + p*16 + f: each partition reads one contiguous
    # 16 KB run per round and writes one contiguous 512 B run.
    x_v = x_d[:, :].rearrange("(r p f) d -> r p f d", r=N_R, p=128, f=J)
    y_v = y_d[:, :].rearrange("(r p f) c -> r p f c", r=N_R, p=128, f=J)
    # p-OUTER view of y: y_vp[:, r0:r1] is a single multi-round store
    # AP ([128, k rounds, J, 8] -- per partition a (r, f) 2D walk), so
    # one finalize batch stores with ONE HWDGE dma_start.
    y_vp = y_d[:, :].rearrange("(r p f) c -> p r f c", r=N_R, p=128, f=J)

    with TileContext(nc) as tc:
        with (
            tc.tile_pool(name="consts", bufs=1) as consts,
            tc.tile_pool(name="x0p", bufs=1) as x0p,
            tc.tile_pool(name="xcp", bufs=8) as xcp,
            tc.tile_pool(name="xtp", bufs=3) as xtp,
            tc.tile_pool(name="h1p", bufs=3) as h1p,
            tc.tile_pool(name="ytp", bufs=3) as ytp,
            tc.tile_pool(name="sqp", bufs=2) as sqp,
            tc.tile_pool(name="accp", bufs=1) as accp,
            tc.tile_pool(name="yout", bufs=1) as yout,
            tc.tile_pool(name="pp", bufs=3, space="PSUM") as pp,
            tc.tile_pool(name="pp2", bufs=3, space="PSUM") as pp2,
        ):
            wp = consts.tile([128, 32], f32)
            nc.sync.dma_start(out=wp, in_=wp_d[:, :])
            wb = consts.tile([128, 288], bf16)
            nc.sync.dma_start(out=wb, in_=wb_d[:, :])
            w1t = wb[:, 0:DIM]
            w2t = wb[:, DIM : DIM + 32]
            b1c = wp[:, 0:1]
            aux = wp[:, 8:32]

            # finalize batches: the last batch's finalize is EMITTED
            # after round 15's transpose+stats (see one_pass), so the
            # tail DVE stream is [yt15, sq15, red15, one rsqrt chain,
            # one scale, one store] with nothing queued ahead of it.
            BATCHES = [(0, 8), (8, 14), (14, 16)]

            # split accumulators per finalize batch: no shared tile
            # between in-flight rounds and a draining finalize.
            cent_b = [
                accp.tile([128, (hi - lo) * 128], f32, name=f"cent{b}",
                          tag=f"cent{b}")
                for b, (lo, hi) in enumerate(BATCHES)
            ]
            ssq_b = [
                accp.tile([128, (hi - lo) * 16], f32, name=f"ssq{b}",
                          tag=f"ssq{b}")
                for b, (lo, hi) in enumerate(BATCHES)
            ]

            def batch_of(r):
                for b, (lo, hi) in enumerate(BATCHES):
                    if lo <= r < hi:
                        return b, r - lo
                raise AssertionError(r)

            def dma_only_pass():
                for r in range(N_R):
                    xc = xcp.tile([128, J, DIM], bf16, tag="xc")
                    nc.gpsimd.dma_start(out=xc, in_=x_v[r])
                    y_t = yout.tile([128, J, 8], f32, tag="y_t")
                    nc.vector.tensor_copy(out=y_t[:, 0:1, :], in_=xc[:, 0:1, 0:8])
                    nc.sync.dma_start(out=y_v[r], in_=y_t)

            rstd_b = [
                accp.tile([128, (hi - lo) * 16], f32, name=f"rstd{b}",
                          tag=f"rstd{b}")
                for b, (lo, hi) in enumerate(BATCHES)
            ]

            def finalize(b):
                # rstd = rsqrt(ssq/8 + eps) via magic-constant seed
                # (int ops on GpSimd) + one DVE Newton step (~0.2% max
                # err, in the noise next to bf16's 3.5e-3): no ACT
                # involvement, so the Gelu table is NEVER switched out
                # (the old ACT Sqrt cost 2 x ~2.7us Gelu<->Sqrt table
                # loads per finalize, parked in the ACT stream right
                # between two GELUs that mm2 was waiting on).
                r_lo, r_hi = BATCHES[b]
                nr = r_hi - r_lo
                n = nr * 16
                i32 = mybir.dt.int32
                v = sqp.tile([128, n], f32, tag="vv")
                nc.vector.tensor_scalar(
                    out=v, in0=ssq_b[b], scalar1=1.0 / OUT, scalar2=EPS,
                    op0=ALU.mult, op1=ALU.add,
                )
                # y0 bits = 0x5f3759df - (v_bits >> 1)
                #         = ((v_bits >> 1) ^ -1) + 0x5f3759e0
                # (on DVE: walrus rejects TensorScalar on Pool/GpSimd)
                t = sqp.tile([128, n], i32, tag="t0")
                nc.vector.tensor_scalar(
                    out=t, in0=v.bitcast(i32), scalar1=1, scalar2=-1,
                    op0=ALU.logical_shift_right, op1=ALU.bitwise_xor,
                )
                nc.vector.tensor_scalar(
                    out=t, in0=t, scalar1=0x5F3759E0, scalar2=None,
                    op0=ALU.add,
                )
                y0 = t.bitcast(f32)
                a = sqp.tile([128, n], f32, tag="na")
                nc.vector.tensor_tensor(out=a, in0=y0, in1=y0, op=ALU.mult)
                nc.vector.tensor_tensor(out=a, in0=v, in1=a, op=ALU.mult)
                nc.vector.tensor_scalar(
                    out=a, in0=a, scalar1=-0.5, scalar2=1.5,
                    op0=ALU.mult, op1=ALU.add,
                )
                nc.vector.tensor_tensor(
                    out=rstd_b[b], in0=y0, in1=a, op=ALU.mult
                )
                # ---- merged scale + ONE multi-round store for the
                # whole batch: one DVE TT (nr*128 cols) replaces nr
                # separate 128-col TTs (saves the per-op fixed cost and
                # shrinks the DVE tail backlog), and the p-outer y view
                # makes the nr-round store a single HWDGE dma_start ----
                m = nr * 16
                y_t = yout.tile([128, nr, J, 8], f32, tag=f"y_t{b}")
                y_tv = y_t.rearrange("p r j c -> p (r j) c")
                cent_v = cent_b[b].rearrange("p (m c) -> p m c", c=8)
                rs = rstd_b[b].rearrange("p (m c) -> p m c", c=1)
                rs = rs.broadcast_to([128, m, 8])
                nc.vector.tensor_tensor(
                    out=y_tv, in0=cent_v, in1=rs, op=ALU.mult
                )
                if use_gamma:
                    gm = aux[:, 8:16].rearrange(
                        "p (j c) -> p j c", j=1
                    ).broadcast_to([128, m, 8])
                    nc.vector.tensor_tensor(
                        out=y_tv, in0=y_tv, in1=gm, op=ALU.mult
                    )
                if use_beta:
                    bt = aux[:, 16:24].rearrange(
                        "p (j c) -> p j c", j=1
                    ).broadcast_to([128, m, 8])
                    nc.vector.tensor_tensor(
                        out=y_tv, in0=y_tv, in1=bt, op=ALU.add
                    )
                nc.sync.dma_start(out=y_vp[:, r_lo:r_hi], in_=y_t)

            def load_x(r):
                # ---- SWDGE cast-DMA: f32 HBM rows -> bf16 SBUF,
                # token-major; the SDMA engines downcast inline ----
                xc = xcp.tile([128, J, DIM], bf16, tag="xc")
                nc.gpsimd.dma_start(out=xc, in_=x_v[r])
                return xc

            def transpose_x(xc):
                # ---- 32x32 block transpose to feature-major, on PACKED
                # u32 pairs: DVE transpose is element-rate-limited
                # (~1.05ns/col regardless of width), so transposing
                # bf16 pairs as one i32 element halves DVE time.
                # u32 col c32 = 32*db32 + b holds features (2c32, 2c32+1),
                # so xt[32P+a, j, 64*db32 + 2b + e] (bf16 view)
                #   = x[token r*2048 + j*128 + 32P + b, d = 64*db32+2a+e]
                # and the PE reads each (db32, e) slice as a stride-2 AP
                # (per-column partition-parallel fetch; stride-free cost).
                # TWO [128, 8, 128] ops, NOT one [128, 16, 128]: the
                # merged op was tried and measured 2312ns vs 2x1042ns
                # -- the half-round shape is the faster DVE mode, the
                # per-op overhead is negative.  Keep the split.
                i32 = mybir.dt.int32
                xt = xtp.tile([128, J, DIM // 2], i32, tag="xt")
                xci = xc.bitcast(i32)
                for q in range(2):
                    w = J // 2
                    nc.vector.transpose(
                        out=xt[:, w * q : w * (q + 1), :],
                        in_=xci[:, w * q : w * (q + 1), :],
                    )
                return xt

            yts = {}

            def stats(r):
                b, i = batch_of(r)
                yt = yts.pop(r)
                cent = cent_b[b][:, i * 128 : (i + 1) * 128].rearrange(
                    "p (j c) -> p j c", c=8
                )
                # whole yt->cent->sq->reduce chain on DVE: it has ~2us
                # of slack since the packed transpose, and keeping the
                # chain single-engine removes two cross-engine hops of
                # latency per round (dominant in the post-DMA drain)
                mu = yt[:, :, 8:9].broadcast_to([128, J, 8])
                nc.vector.tensor_tensor(
                    out=cent, in0=yt[:, :, 0:8], in1=mu, op=ALU.subtract
                )
                if use_b2c:
                    b2c = aux[:, 0:8].rearrange(
                        "p (j c) -> p j c", j=1
                    ).broadcast_to([128, J, 8])
                    nc.vector.tensor_tensor(
                        out=cent, in0=cent, in1=b2c, op=ALU.add
                    )
                sq = sqp.tile([128, 128], f32, tag="sq")
                nc.vector.tensor_tensor(
                    out=sq,
                    in0=cent_b[b][:, i * 128 : (i + 1) * 128],
                    in1=cent_b[b][:, i * 128 : (i + 1) * 128],
                    op=ALU.mult,
                )
                nc.vector.reduce_sum(
                    out=ssq_b[b][:, i * 16 : (i + 1) * 16],
                    in_=sq.rearrange("p (j c) -> p j c", c=8),
                    axis=mybir.AxisListType.X,
                )

            def mm1_round(r, xts, splits=1):
                # bf16 view of the u32-transposed tile:
                # col (db32, b, e) = 64*db32 + 2b + e
                xt_b = xts.pop(r).bitcast(bf16).rearrange(
                    "p j (db b e) -> p j db e b", db=4, b=32, e=2
                )
                # ---- mm1: 4 diagonal streams, one full psum bank;
                # 8 K-steps k = 2*db32 + e over the packed layout.
                # splits>1 (round 0 only) chops the round into j-column
                # groups so the first matmuls issue as soon as the first
                # sub-load's transpose lands, shortening the pipeline
                # fill and warming the PE's HAM clock early ----
                ps = pp.tile([128, 512], f32, name="ps", tag="ps")
                jw = J // splits
                for s in range(splits):
                    for db in range(NDB):
                        db32, e = divmod(db, 2)
                        for P in range(4):
                            nc.tensor.matmul(
                                out=ps[
                                    32 * P : 32 * P + 32,
                                    s * jw * 32 : (s + 1) * jw * 32,
                                ],
                                lhsT=w1t[
                                    32 * P : 32 * P + 32,
                                    32 * db : 32 * db + 32,
                                ],
                                rhs=xt_b[
                                    32 * P : 32 * P + 32,
                                    s * jw : (s + 1) * jw,
                                    db32, e, :,
                                ],
                                start=(db == 0),
                                stop=(db == NDB - 1),
                                tile_position=(32 * P, 32 * P),
                                skip_group_check=True,
                            )
                # ---- exact GELU (erf) on all 128 lanes, + b1,
                # bf16 out feeding mm2; runs on ACT while the PE is
                # already into mm1(r+1) ----
                h1 = h1p.tile([128, 512], bf16, tag="h1")
                nc.scalar.activation(
                    out=h1, in_=ps,
                    func=SIM_ACT_OVERRIDE[0] or AF.Gelu,
                    bias=b1c, scale=1.0,
                )
                return h1

            def mm2_stats_round(r, h1):
                # ---- mm2: 4 diagonal K=8 streams.  EMITTED AFTER
                # mm1(r+1): the PE queue is in-order, so putting mm2(r)
                # right after mm1(r) made the PE sit idle ~1.2-1.5us
                # every round waiting for GELU(r).  One round of skew
                # lets GELU(r) finish while the PE runs mm1(r+1), so
                # mm2(r) issues bubble-free ----
                ps2 = pp2.tile([128, 512], f32, name="ps2", tag="ps2")
                for g in range(4):
                    nc.tensor.matmul(
                        out=ps2[32 * g : 32 * g + 32, :],
                        lhsT=w2t[32 * g : 32 * g + 8, 0:32],
                        rhs=h1[32 * g : 32 * g + 8, :],
                        start=True,
                        stop=True,
                        tile_position=(32 * g, 32 * g),
                        skip_group_check=True,
                    )
                yt = ytp.tile([128, J, 32], f32, tag="yt")
                # ONE op, not two halves: splitting was tried (the xt
                # transposes measured faster as halves) but yt reads
                # PSUM, and each op pays the ~120-cycle PSUM access
                # latency -- halves measured 2 x ~410ns vs 674ns.
                nc.vector.transpose(out=yt, in_=ps2[:, :])
                # yt[p, j, c]: c 0..7 = h2 channels, c 8 = mean
                yts[r] = yt
                stats(r)
                # the LAST batch's finalize is emitted by one_pass
                # after round 15's stats, so round-15's yt/stats ops
                # aren't queued on DVE behind an earlier batch's
                # rsqrt/scale chain in the kernel tail
                for fb, (lo, hi) in enumerate(BATCHES):
                    if r == hi - 1 and hi < N_R:
                        finalize(fb)

            def one_pass():
              if variant == "dmaonly":
                  dma_only_pass()
                  return
              # software-pipelined: loads and transposes for round r+1
              # are EMITTED before round r's mm1, and mm2/stats run one
              # round behind mm1 (see mm2_stats_round).
              xts = {}
              h1s = {}
              # round 0 arrives as 4 independent sub-loads, each with
              # its own tile so its transpose waits only on its own
              # bytes -- first mm1 matmuls issue ~4us sooner.
              i32 = mybir.dt.int32
              SUBJ = J // 4
              x0s = []
              for s in range(4):
                  x0 = x0p.tile([128, SUBJ, DIM], bf16, tag=f"x0_{s}")
                  nc.gpsimd.dma_start(
                      out=x0, in_=x_v[0][:, s * SUBJ : (s + 1) * SUBJ, :]
                  )
                  x0s.append(x0)
              xcs = {q: load_x(q) for q in range(1, 5)}
              xt0 = xtp.tile([128, J, DIM // 2], i32, tag="xt")
              for s in range(4):
                  nc.vector.transpose(
                      out=xt0[:, s * SUBJ : (s + 1) * SUBJ, :],
                      in_=x0s[s].bitcast(i32),
                  )
              xts[0] = xt0
              for r in range(N_R):
                  if r + 5 < N_R:
                      xcs[r + 5] = load_x(r + 5)
                  if r + 1 < N_R:
                      xts[r + 1] = transpose_x(xcs.pop(r + 1))
                  h1s[r] = mm1_round(r, xts, splits=(4 if r == 0 else 1))
                  if r >= 1:
                      mm2_stats_round(r - 1, h1s.pop(r - 1))
              mm2_stats_round(N_R - 1, h1s.pop(N_R - 1))
              finalize(len(BATCHES) - 1)

            for _rep in range(repeat):
                one_pass()

    nc.compile()
    _BUILD_CACHE[key] = nc
    return nc


def prep_inputs(x, W1, b1, W2, b2, gamma, beta):
    """Host-side prep: shard x, lay out the tiny weights for the kernel."""
    import ml_dtypes

    x = np.ascontiguousarray(np.asarray(x, dtype=np.float32)).reshape(TOK_TOTAL, DIM)
    W1 = np.asarray(W1, dtype=np.float32)
    b1 = np.asarray(b1, dtype=np.float32)
    W2 = np.asarray(W2, dtype=np.float32)
    b2 = np.asarray(b2, dtype=np.float32)
    gamma = np.asarray(gamma, dtype=np.float32)
    beta = np.asarray(beta, dtype=np.float32)

    # packed-pair layout: K-step k = 2*db32 + e contracts features
    # d = 64*db32 + 2a + e at partition 32P+a, so
    # w1t[32P+a, 32k+o] = W1[o, 64*(k//2) + 2a + (k%2)], replicated per P
    kk = np.arange(NDB)
    aa = np.arange(32)
    dmat = 64 * (kk[:, None] // 2) + 2 * aa[None, :] + (kk[:, None] % 2)
    w1g = np.zeros((32, NDB, 32), np.float32)            # [a, k, oslot]
    w1g[:, :, :OUT] = W1[:, dmat].transpose(2, 1, 0)     # [o,k,a]->[a,k,o]
    w1t = np.tile(w1g.reshape(32, DIM), (4, 1))

    # w2t9[32g+o, m] = W2[m, o] (o < 8); col 8 = mean over rows of W2,
    # replicated into each 32-partition group
    w2t9 = np.zeros((32, 32), np.float32)
    w2t9[:OUT, :OUT] = W2.T
    w2t9[:OUT, 8] = W2.mean(axis=0)
    w2rep = np.tile(w2t9, (4, 1))

    use_b2c = bool(np.any(b2 != 0.0))
    use_gamma = bool(np.any(gamma != 1.0))
    use_beta = bool(np.any(beta != 0.0))

    wpackb = np.zeros((128, 288), ml_dtypes.bfloat16)
    wpackb[:, 0:DIM] = w1t.astype(ml_dtypes.bfloat16)
    wpackb[:, DIM : DIM + 32] = w2rep.astype(ml_dtypes.bfloat16)

    wpack = np.zeros((128, 32), np.float32)
    b1full = np.zeros((128,), np.float32)
    for g in range(4):
        b1full[32 * g : 32 * g + OUT] = b1
    wpack[:, 0] = b1full
    wpack[:, 8:16] = (b2 - b2.mean())[None, :]
    wpack[:, 16:24] = gamma[None, :]
    wpack[:, 24:32] = beta[None, :]

    in_maps = []
    for k in range(N_CORES):
        m = {
            "x": np.ascontiguousarray(x[k * TOK_CORE : (k + 1) * TOK_CORE]),
            "wpack": wpack,
            "wpackb": wpackb,
        }
        in_maps.append(m)
    flags = dict(use_b2c=use_b2c, use_gamma=use_gamma, use_beta=use_beta)
    return in_maps, flags


def run(x, W1, b1, W2, b2, gamma, beta, trace=False, variant="full", **kw):
    from concourse.bass_utils import run_bass_kernel_spmd

    kw.pop("mm_f32r", None)
    in_maps, flags = prep_inputs(x, W1, b1, W2, b2, gamma, beta)
    nc = build_kernel(variant=variant, **flags)
    res = run_bass_kernel_spmd(
        nc, in_maps, core_ids=list(range(N_CORES)), trace=trace, **kw
    )
    y = np.concatenate([res.results[k]["y"] for k in range(N_CORES)], axis=0)
    return y.reshape(B, T, OUT).astype(np.float32), res


def kernel(x, W1, b1, W2, b2, gamma, beta):
    y, _ = run(x, W1, b1, W2, b2, gamma, beta)
    return y



# revision 57
# speedup vs baseline: 1.0493x; 1.0493x over previous
"""Trainium2 Bass kernel for nn_BottleneckFFN.

Computes y = LayerNorm(GELU(x @ W1.T + b1) @ W2.T + b2) * gamma + beta
for x of shape (128, 2048, 256), W1 (8, 256), W2 (8, 8), LN over the
trailing 8 channels.  Pure data parallel over 8 NeuronCores: the
128*2048 = 262144 token rows are split into 8 shards of 32768 tokens;
the tiny weights are replicated.

Per-core dataflow (per round of 2048 tokens), software-pipelined with a
1-round skew (loads/transposes for round r+1 are emitted before round
r's matmul stages):
  1. SWDGE cast-DMA (nc.gpsimd.dma_start with f32 in / bf16 out): the
     SDMA engines downcast inline, so 2 MB of HBM reads land directly
     as a 1 MB token-major bf16 tile ([128 part, 16 tiles, 256]).
     This deletes the old ACT(3.5us)+GpSimd(1.9us) per-round cast
     stage entirely -- the v1 trace showed those casts (plus the DVE
     transpose) starving the PE into HAM-cold oscillation (45%
     throttle time) and stretching the post-DMA drain to ~50us.
     bf16 matmuls keep full PE speed without float32r's
     psum-partition-base-0 restriction.  Measured rel err 3.5e-3
     (gate 2e-2).  Descriptor generation (~1us/round) runs on the
     GpSimd engine, which has nothing else to do.
  2. Two DVE 32x32 block transposes to feature-major per 32-partition
     group, on the bf16 tile BITCAST TO i32 so each transposed element
     is a packed pair of adjacent features: DVE transpose is
     element-rate-limited (~1.05 ns/col regardless of width), so the
     i32 packing halves DVE transpose time.  mm1 then reads even/odd
     features as stride-2 bf16 APs (per-column partition-parallel
     fetch makes PE cost stride-independent), with W1 reordered
     host-side to match (K-step k = 2*db32 + e contracts
     d = 64*db32 + 2a + e at partition 32P+a).
  3. mm1: 8 d-blocks x 4 concurrent diagonal K=32 bf16 matmuls
     (tile_position (32P, 32P)) accumulate x @ W1.T into ONE psum bank
     as [128, 512]: token group P's channels land at partitions
     32P..32P+32 (same-bank different-partition writes are safe).
     pp bufs=3 (and pp2 bufs=3; 6 of 8 banks) buffers the bank so
     mm1(r+1..r+2) overlap GELU(r).
  4. Exact GELU over all 128 lanes, b1 fused as per-partition bias,
     bf16 output feeding mm2 directly.  With the cast gone, GELU
     (~0.6us) is ACT's only steady-state work.
  5. mm2: 4 concurrent diagonal K=8 bf16 matmuls with a 32-col
     stationary whose col 8 is mean(W2 rows), so the per-token LN mean
     falls out of the matmul; fresh double-buffered psum bank.
  6. One DVE block-transpose back to token-major; centered = h2 - mu,
     Square, grouped reduce, all on DVE (~4.3us/round total DVE,
     under the 5.2us DMA round).
  7. Finalize per batch (rounds 0-8 / 8-14 / 14-16; the last batch's
     finalize is EMITTED after round 15's transpose+stats, so nothing
     queues ahead of the tail DVE chain):
     rstd = rsqrt(ssq/8 + eps) via magic-constant seed + one DVE
     Newton step -- table-free, so the ACT Gelu table is NEVER
     swapped (the old ACT Sqrt cost 2 x ~2.7us Gelu<->Sqrt table
     loads per finalize, parked between two GELUs that mm2 was
     waiting on).  The whole batch is then scaled in ONE wide DVE
     tensor_tensor (cent * rstd-broadcast over nr*128 cols) and
     stored with ONE multi-round HWDGE dma_start through a p-outer
     view of y (per partition the batch is a (round, f) 2D walk), so
     per-round scale/store op overhead and the DVE tail backlog both
     collapse; stores go out on the idle SP engine so nothing shares
     the GpSimd SWDGE descgen or the ACT GELU stream.

Key scheduling facts learned from traces (see git-less history in
kernel_v*.py):
  - The PE queue is in-order: mm2(r) emitted right after mm1(r) made
    the PE idle ~1.2-1.5us/round waiting on GELU(r).  Emitting mm2 one
    round late (after mm1(r+1)) removed the bubble: 126.8 -> 113.7us.
  - Round 0 is loaded as 4 independent sub-tiles with per-sub
    transposes and j-split mm1 so first matmuls issue at ~12.5us
    instead of ~15.5 (shorter fill, earlier HAM warm-up).
  - Steady state is at the HBM read roofline (~4.9-5.2us/round,
    ~430 GB/s effective); DVE is the closest compute engine
    (~4.6us/round).  Moving stats/scales to GpSimd (2-input TT) or
    folding the LN-mean into a centered mm2 stationary with persistent
    yt tiles both REGRESSED >13us on HW (GpSimd elementwise is far
    slower than its 2x-of-DVE billing; the persistent-yt variant
    re-cooled the PE) -- both were reverted.

v1 (HWDGE f32 loads + on-engine casts) measured 136.8us: loads ran at
~425 GB/s and finished by t=88us, but the cast+transpose+stats chain
could only retire rounds at ~6-10us in the drain, and the HAM-cold PE
(545ns/matmul = 1.2 GHz) stretched mm1.  Removing the cast (v2,
126.8us), the table-free finalize (v3), the mm2 bubble fix (v4,
113.7us), the round-0 sub-split (v5, 112.9us) and the merged
batch-wide scale+store (v8, 112.4us) landed at ~6.5us preamble +
~6us fill + ~80us of DMA-roofline rounds + ~13us tail+barrier.
Beware ~13% run-to-run drift on this part (HAM clock-gate phase +
cross-core HBM contention): identical binaries measured 112.9 and
128.7 in one session.  PE "heater" tricks were tried twice and
reverted: a 3.7us burst of K=32 matmuls at t=7.7-11.5 (memset-fed, so
it really did run in the fill window) did NOT trip HAM's un-throttle
-- the first K=8/8 event still came at t=33us -- so the activity
monitor evidently discounts low-array-utilization matmuls (1-4 of 16
sub-arrays; even the real diagonal mm1 only lights 4).  The warm/cold
lottery cannot be steered from the instruction stream; don't burn PE
time trying.
"""

import os
import sys

import numpy as np

if not any(os.path.isdir(os.path.join(p, "concourse")) for p in sys.path if p):
    for _cand in ("/opt/trn_rl_repo", "/root/.axon_site/_ro/trn_rl_repo"):
        if os.path.isdir(os.path.join(_cand, "concourse")):
            sys.path.insert(0, _cand)
            break

N_CORES = 8
DIM, OUT = 256, 8
B, T = 128, 2048
TOK_TOTAL = B * T
TOK_CORE = TOK_TOTAL // N_CORES  # 32768
R_TOK = 2048                     # tokens per round
N_R = TOK_CORE // R_TOK          # 16 rounds
J = R_TOK // 128                 # 16 [128, 256] tiles per round
JH = J // 2                      # 8 tiles per half-round
NDB = DIM // 32                  # 8 d-blocks of 32
EPS = 1e-5

_BUILD_CACHE = {}

# CoreSim doesn't implement Gelu; sim_test.py swaps in Tanh (and mirrors
# it in its numpy reference) to validate dataflow/layout off-hardware.
SIM_ACT_OVERRIDE = [None]


def build_kernel(use_b2c=False, use_gamma=False, use_beta=False,
                 repeat=1, variant="full"):
    """Build the per-core Bass program. Returns the compiled Bacc object."""
    key = (use_b2c, use_gamma, use_beta, repeat, variant,
           str(SIM_ACT_OVERRIDE[0]))
    if key in _BUILD_CACHE:
        return _BUILD_CACHE[key]

    import concourse.bacc as bacc
    import concourse.mybir as mybir
    from concourse.tile import TileContext

    f32 = mybir.dt.float32
    bf16 = mybir.dt.bfloat16
    AF = mybir.ActivationFunctionType
    ALU = mybir.AluOpType

    nc = bacc.Bacc("TRN2")
    x_d = nc.dram_tensor("x", [TOK_CORE, DIM], f32, kind="ExternalInput")
    # f32 consts: col 0 b1 (replicated per 32-group), 8:16 b2-mean(b2),
    # 16:24 gamma, 24:32 beta
    wp_d = nc.dram_tensor("wpack", [128, 32], f32, kind="ExternalInput")
    # bf16 consts: cols 0:256 w1t blocks, 256:288 w2t9 (replicated per
    # 32-group)
    wb_d = nc.dram_tensor("wpackb", [128, 288], bf16, kind="ExternalInput")
    y_d = nc.dram_tensor("y", [TOK_CORE, OUT], f32, kind="ExternalOutput")

    # token t = r*2048 + p*16 + f: each partition reads one contiguous
    # 16 KB run per round and writes one contiguous 512 B run.
    x_v = x_d[:, :].rearrange("(r p f) d -> r p f d", r=N_R, p=128, f=J)
    y_v = y_d[:, :].rearrange("(r p f) c -> r p f c", r=N_R, p=128, f=J)
    # p-OUTER view of y: y_vp[:, r0:r1] is a single multi-round store
    # AP ([128, k rounds, J, 8] -- per partition a (r, f) 2D walk), so
    # one finalize batch stores with ONE HWDGE dma_start.
    y_vp = y_d[:, :].rearrange("(r p f) c -> p r f c", r=N_R, p=128, f=J)

    with TileContext(nc) as tc:
        with (
            tc.tile_pool(name="consts", bufs=1) as consts,
            tc.tile_pool(name="x0p", bufs=1) as x0p,
            tc.tile_pool(name="xcp", bufs=8) as xcp,
            tc.tile_pool(name="xtp", bufs=3) as xtp,
            tc.tile_pool(name="h1p", bufs=3) as h1p,
            tc.tile_pool(name="ytp", bufs=3) as ytp,
            tc.tile_pool(name="sqp", bufs=2) as sqp,
            tc.tile_pool(name="accp", bufs=1) as accp,
            tc.tile_pool(name="yout", bufs=1) as yout,
            tc.tile_pool(name="pp", bufs=3, space="PSUM") as pp,
            tc.tile_pool(name="pp2", bufs=3, space="PSUM") as pp2,
        ):
            wp = consts.tile([128, 32], f32)
            nc.sync.dma_start(out=wp, in_=wp_d[:, :])
            wb = consts.tile([128, 288], bf16)
            nc.sync.dma_start(out=wb, in_=wb_d[:, :])
            w1t = wb[:, 0:DIM]
            w2t = wb[:, DIM : DIM + 32]
            b1c = wp[:, 0:1]
            aux = wp[:, 8:32]

            # finalize batches: the last batch's finalize is EMITTED
            # after round 15's transpose+stats (see one_pass), so the
            # tail DVE stream is [yt15, sq15, red15, one rsqrt chain,
            # one scale, one store] with nothing queued ahead of it.
            BATCHES = [(0, 8), (8, 14), (14, 16)]

            # split accumulators per finalize batch: no shared tile
            # between in-flight rounds and a draining finalize.
            cent_b = [
                accp.tile([128, (hi - lo) * 128], f32, name=f"cent{b}",
                          tag=f"cent{b}")
                for b, (lo, hi) in enumerate(BATCHES)
            ]
            ssq_b = [
                accp.tile([128, (hi - lo) * 16], f32, name=f"ssq{b}",
                          tag=f"ssq{b}")
                for b, (lo, hi) in enumerate(BATCHES)
            ]

            def batch_of(r):
                for b, (lo, hi) in enumerate(BATCHES):
                    if lo <= r < hi:
                        return b, r - lo
                raise AssertionError(r)

            def dma_only_pass():
                for r in range(N_R):
                    xc = xcp.tile([128, J, DIM], bf16, tag="xc")
                    nc.gpsimd.dma_start(out=xc, in_=x_v[r])
                    y_t = yout.tile([128, J, 8], f32, tag="y_t")
                    nc.vector.tensor_copy(out=y_t[:, 0:1, :], in_=xc[:, 0:1, 0:8])
                    nc.sync.dma_start(out=y_v[r], in_=y_t)

            rstd_b = [
                accp.tile([128, (hi - lo) * 16], f32, name=f"rstd{b}",
                          tag=f"rstd{b}")
                for b, (lo, hi) in enumerate(BATCHES)
            ]

            def finalize(b):
                # rstd = rsqrt(ssq/8 + eps) via magic-constant seed
                # (int ops on GpSimd) + one DVE Newton step (~0.2% max
                # err, in the noise next to bf16's 3.5e-3): no ACT
                # involvement, so the Gelu table is NEVER switched out
                # (the old ACT Sqrt cost 2 x ~2.7us Gelu<->Sqrt table
                # loads per finalize, parked in the ACT stream right
                # between two GELUs that mm2 was waiting on).
                r_lo, r_hi = BATCHES[b]
                nr = r_hi - r_lo
                n = nr * 16
                i32 = mybir.dt.int32
                v = sqp.tile([128, n], f32, tag="vv")
                nc.vector.tensor_scalar(
                    out=v, in0=ssq_b[b], scalar1=1.0 / OUT, scalar2=EPS,
                    op0=ALU.mult, op1=ALU.add,
                )
                # y0 bits = 0x5f3759df - (v_bits >> 1)
                #         = ((v_bits >> 1) ^ -1) + 0x5f3759e0
                # (on DVE: walrus rejects TensorScalar on Pool/GpSimd)
                t = sqp.tile([128, n], i32, tag="t0")
                nc.vector.tensor_scalar(
                    out=t, in0=v.bitcast(i32), scalar1=1, scalar2=-1,
                    op0=ALU.logical_shift_right, op1=ALU.bitwise_xor,
                )
                nc.vector.tensor_scalar(
                    out=t, in0=t, scalar1=0x5F3759E0, scalar2=None,
                    op0=ALU.add,
                )
                y0 = t.bitcast(f32)
                a = sqp.tile([128, n], f32, tag="na")
                nc.vector.tensor_tensor(out=a, in0=y0, in1=y0, op=ALU.mult)
                nc.vector.tensor_tensor(out=a, in0=v, in1=a, op=ALU.mult)
                nc.vector.tensor_scalar(
                    out=a, in0=a, scalar1=-0.5, scalar2=1.5,
                    op0=ALU.mult, op1=ALU.add,
                )
                nc.vector.tensor_tensor(
                    out=rstd_b[b], in0=y0, in1=a, op=ALU.mult
                )
                # ---- merged scale + ONE multi-round store for the
                # whole batch: one DVE TT (nr*128 cols) replaces nr
                # separate 128-col TTs (saves the per-op fixed cost and
                # shrinks the DVE tail backlog), and the p-outer y view
                # makes the nr-round store a single HWDGE dma_start ----
                m = nr * 16
                y_t = yout.tile([128, nr, J, 8], f32, tag=f"y_t{b}")
                y_tv = y_t.rearrange("p r j c -> p (r j) c")
                cent_v = cent_b[b].rearrange("p (m c) -> p m c", c=8)
                rs = rstd_b[b].rearrange("p (m c) -> p m c", c=1)
                rs = rs.broadcast_to([128, m, 8])
                nc.vector.tensor_tensor(
                    out=y_tv, in0=cent_v, in1=rs, op=ALU.mult
                )
                if use_gamma:
                    gm = aux[:, 8:16].rearrange(
                        "p (j c) -> p j c", j=1
                    ).broadcast_to([128, m, 8])
                    nc.vector.tensor_tensor(
                        out=y_tv, in0=y_tv, in1=gm, op=ALU.mult
                    )
                if use_beta:
                    bt = aux[:, 16:24].rearrange(
                        "p (j c) -> p j c", j=1
                    ).broadcast_to([128, m, 8])
                    nc.vector.tensor_tensor(
                        out=y_tv, in0=y_tv, in1=bt, op=ALU.add
                    )
                nc.sync.dma_start(out=y_vp[:, r_lo:r_hi], in_=y_t)

            def load_x(r):
                # ---- SWDGE cast-DMA: f32 HBM rows -> bf16 SBUF,
                # token-major; the SDMA engines downcast inline ----
                xc = xcp.tile([128, J, DIM], bf16, tag="xc")
                nc.gpsimd.dma_start(out=xc, in_=x_v[r])
                return xc

            def transpose_x(xc):
                # ---- 32x32 block transpose to feature-major, on PACKED
                # u32 pairs: DVE transpose is element-rate-limited
                # (~1.05ns/col regardless of width), so transposing
                # bf16 pairs as one i32 element halves DVE time.
                # u32 col c32 = 32*db32 + b holds features (2c32, 2c32+1),
                # so xt[32P+a, j, 64*db32 + 2b + e] (bf16 view)
                #   = x[token r*2048 + j*128 + 32P + b, d = 64*db32+2a+e]
                # and the PE reads each (db32, e) slice as a stride-2 AP
                # (per-column partition-parallel fetch; stride-free cost).
                # TWO [128, 8, 128] ops, NOT one [128, 16, 128]: the
                # merged op was tried and measured 2312ns vs 2x1042ns
                # -- the half-round shape is the faster DVE mode, the
                # per-op overhead is negative.  Keep the split.
                i32 = mybir.dt.int32
                xt = xtp.tile([128, J, DIM // 2], i32, tag="xt")
                xci = xc.bitcast(i32)
                for q in range(2):
                    w = J // 2
                    nc.vector.transpose(
                        out=xt[:, w * q : w * (q + 1), :],
                        in_=xci[:, w * q : w * (q + 1), :],
                    )
                return xt

            yts = {}

            def stats(r):
                b, i = batch_of(r)
                yt = yts.pop(r)
                cent = cent_b[b][:, i * 128 : (i + 1) * 128].rearrange(
                    "p (j c) -> p j c", c=8
                )
                # whole yt->cent->sq->reduce chain on DVE: it has ~2us
                # of slack since the packed transpose, and keeping the
                # chain single-engine removes two cross-engine hops of
                # latency per round (dominant in the post-DMA drain)
                mu = yt[:, :, 8:9].broadcast_to([128, J, 8])
                nc.vector.tensor_tensor(
                    out=cent, in0=yt[:, :, 0:8], in1=mu, op=ALU.subtract
                )
                if use_b2c:
                    b2c = aux[:, 0:8].rearrange(
                        "p (j c) -> p j c", j=1
                    ).broadcast_to([128, J, 8])
                    nc.vector.tensor_tensor(
                        out=cent, in0=cent, in1=b2c, op=ALU.add
                    )
                sq = sqp.tile([128, 128], f32, tag="sq")
                nc.vector.tensor_tensor(
                    out=sq,
                    in0=cent_b[b][:, i * 128 : (i + 1) * 128],
                    in1=cent_b[b][:, i * 128 : (i + 1) * 128],
                    op=ALU.mult,
                )
                nc.vector.reduce_sum(
                    out=ssq_b[b][:, i * 16 : (i + 1) * 16],
                    in_=sq.rearrange("p (j c) -> p j c", c=8),
                    axis=mybir.AxisListType.X,
                )

            def mm1_round(r, xts, splits=1):
                # bf16 view of the u32-transposed tile:
                # col (db32, b, e) = 64*db32 + 2b + e
                xt_b = xts.pop(r).bitcast(bf16).rearrange(
                    "p j (db b e) -> p j db e b", db=4, b=32, e=2
                )
                # ---- mm1: 4 diagonal streams, one full psum bank;
                # 8 K-steps k = 2*db32 + e over the packed layout.
                # splits>1 (round 0 only) chops the round into j-column
                # groups so the first matmuls issue as soon as the first
                # sub-load's transpose lands, shortening the pipeline
                # fill and warming the PE's HAM clock early ----
                ps = pp.tile([128, 512], f32, name="ps", tag="ps")
                jw = J // splits
                for s in range(splits):
                    for db in range(NDB):
                        db32, e = divmod(db, 2)
                        for P in range(4):
                            nc.tensor.matmul(
                                out=ps[
                                    32 * P : 32 * P + 32,
                                    s * jw * 32 : (s + 1) * jw * 32,
                                ],
                                lhsT=w1t[
                                    32 * P : 32 * P + 32,
                                    32 * db : 32 * db + 32,
                                ],
                                rhs=xt_b[
                                    32 * P : 32 * P + 32,
                                    s * jw : (s + 1) * jw,
                                    db32, e, :,
                                ],
                                start=(db == 0),
                                stop=(db == NDB - 1),
                                tile_position=(32 * P, 32 * P),
                                skip_group_check=True,
                            )
                # ---- exact GELU (erf) on all 128 lanes, + b1,
                # bf16 out feeding mm2; runs on ACT while the PE is
                # already into mm1(r+1) ----
                h1 = h1p.tile([128, 512], bf16, tag="h1")
                nc.scalar.activation(
                    out=h1, in_=ps,
                    func=SIM_ACT_OVERRIDE[0] or AF.Gelu,
                    bias=b1c, scale=1.0,
                )
                return h1

            def mm2_stats_round(r, h1):
                # ---- mm2: 4 diagonal K=8 streams.  EMITTED AFTER
                # mm1(r+1): the PE queue is in-order, so putting mm2(r)
                # right after mm1(r) made the PE sit idle ~1.2-1.5us
                # every round waiting for GELU(r).  One round of skew
                # lets GELU(r) finish while the PE runs mm1(r+1), so
                # mm2(r) issues bubble-free ----
                ps2 = pp2.tile([128, 512], f32, name="ps2", tag="ps2")
                for g in range(4):
                    nc.tensor.matmul(
                        out=ps2[32 * g : 32 * g + 32, :],
                        lhsT=w2t[32 * g : 32 * g + 8, 0:32],
                        rhs=h1[32 * g : 32 * g + 8, :],
                        start=True,
                        stop=True,
                        tile_position=(32 * g, 32 * g),
                        skip_group_check=True,
                    )
                yt = ytp.tile([128, J, 32], f32, tag="yt")
                # ONE op, not two halves: splitting was tried (the xt
                # transposes measured faster as halves) but yt reads
                # PSUM, and each op pays the ~120-cycle PSUM access
                # latency -- halves measured 2 x ~410ns vs 674ns.
                nc.vector.transpose(out=yt, in_=ps2[:, :])
                # yt[p, j, c]: c 0..7 = h2 channels, c 8 = mean
                yts[r] = yt
                stats(r)
                # the LAST batch's finalize is emitted by one_pass
                # after round 15's stats, so round-15's yt/stats ops
                # aren't queued on DVE behind an earlier batch's
                # rsqrt/scale chain in the kernel tail
                for fb, (lo, hi) in enumerate(BATCHES):
                    if r == hi - 1 and hi < N_R:
                        finalize(fb)

            def one_pass():
              if variant == "dmaonly":
                  dma_only_pass()
                  return
              # software-pipelined: loads and transposes for round r+1
              # are EMITTED before round r's mm1, and mm2/stats run one
              # round behind mm1 (see mm2_stats_round).
              xts = {}
              h1s = {}
              # round 0 arrives as 4 independent sub-loads, each with
              # its own tile so its transpose waits only on its own
              # bytes -- first mm1 matmuls issue ~4us sooner.
              i32 = mybir.dt.int32
              SUBJ = J // 4
              x0s = []
              for s in range(4):
                  x0 = x0p.tile([128, SUBJ, DIM], bf16, tag=f"x0_{s}")
                  if s == 0:
                      # first sub-load via HWDGE f32 + ACT cast: both
                      # SP and ACT are idle at kernel start, HWDGE
                      # first-byte is ~0.6us vs SWDGE ~1-2us, and it
                      # skips the Q7 descgen queue behind which the
                      # SWDGE sub-loads serialize -- the first
                      # transpose starts ~1.5us sooner.
                      x0f = x0p.tile([128, SUBJ, DIM], f32, tag="x0f")
                      nc.sync.dma_start(
                          out=x0f, in_=x_v[0][:, 0:SUBJ, :]
                      )
                      nc.scalar.activation(
                          out=x0, in_=x0f, func=AF.Copy,
                          bias=0.0, scale=1.0,
                      )
                  else:
                      nc.gpsimd.dma_start(
                          out=x0, in_=x_v[0][:, s * SUBJ : (s + 1) * SUBJ, :]
                      )
                  x0s.append(x0)
              xcs = {q: load_x(q) for q in range(1, 5)}
              xt0 = xtp.tile([128, J, DIM // 2], i32, tag="xt")
              for s in range(4):
                  nc.vector.transpose(
                      out=xt0[:, s * SUBJ : (s + 1) * SUBJ, :],
                      in_=x0s[s].bitcast(i32),
                  )
              xts[0] = xt0
              for r in range(N_R):
                  if r + 5 < N_R:
                      xcs[r + 5] = load_x(r + 5)
                  if r + 1 < N_R:
                      xts[r + 1] = transpose_x(xcs.pop(r + 1))
                  h1s[r] = mm1_round(r, xts, splits=(4 if r == 0 else 1))
                  if r >= 1:
                      mm2_stats_round(r - 1, h1s.pop(r - 1))
              mm2_stats_round(N_R - 1, h1s.pop(N_R - 1))
              finalize(len(BATCHES) - 1)

            for _rep in range(repeat):
                one_pass()

    nc.compile()
    _BUILD_CACHE[key] = nc
    return nc


def prep_inputs(x, W1, b1, W2, b2, gamma, beta):
    """Host-side prep: shard x, lay out the tiny weights for the kernel."""
    import ml_dtypes

    x = np.ascontiguousarray(np.asarray(x, dtype=np.float32)).reshape(TOK_TOTAL, DIM)
    W1 = np.asarray(W1, dtype=np.float32)
    b1 = np.asarray(b1, dtype=np.float32)
    W2 = np.asarray(W2, dtype=np.float32)
    b2 = np.asarray(b2, dtype=np.float32)
    gamma = np.asarray(gamma, dtype=np.float32)
    beta = np.asarray(beta, dtype=np.float32)

    # packed-pair layout: K-step k = 2*db32 + e contracts features
    # d = 64*db32 + 2a + e at partition 32P+a, so
    # w1t[32P+a, 32k+o] = W1[o, 64*(k//2) + 2a + (k%2)], replicated per P
    kk = np.arange(NDB)
    aa = np.arange(32)
    dmat = 64 * (kk[:, None] // 2) + 2 * aa[None, :] + (kk[:, None] % 2)
    w1g = np.zeros((32, NDB, 32), np.float32)            # [a, k, oslot]
    w1g[:, :, :OUT] = W1[:, dmat].transpose(2, 1, 0)     # [o,k,a]->[a,k,o]
    w1t = np.tile(w1g.reshape(32, DIM), (4, 1))

    # w2t9[32g+o, m] = W2[m, o] (o < 8); col 8 = mean over rows of W2,
    # replicated into each 32-partition group
    w2t9 = np.zeros((32, 32), np.float32)
    w2t9[:OUT, :OUT] = W2.T
    w2t9[:OUT, 8] = W2.mean(axis=0)
    w2rep = np.tile(w2t9, (4, 1))

    use_b2c = bool(np.any(b2 != 0.0))
    use_gamma = bool(np.any(gamma != 1.0))
    use_beta = bool(np.any(beta != 0.0))

    wpackb = np.zeros((128, 288), ml_dtypes.bfloat16)
    wpackb[:, 0:DIM] = w1t.astype(ml_dtypes.bfloat16)
    wpackb[:, DIM : DIM + 32] = w2rep.astype(ml_dtypes.bfloat16)

    wpack = np.zeros((128, 32), np.float32)
    b1full = np.zeros((128,), np.float32)
    for g in range(4):
        b1full[32 * g : 32 * g + OUT] = b1
    wpack[:, 0] = b1full
    wpack[:, 8:16] = (b2 - b2.mean())[None, :]
    wpack[:, 16:24] = gamma[None, :]
    wpack[:, 24:32] = beta[None, :]

    in_maps = []
    for k in range(N_CORES):
        m = {
            "x": np.ascontiguousarray(x[k * TOK_CORE : (k + 1) * TOK_CORE]),
            "wpack": wpack,
            "wpackb": wpackb,
        }
        in_maps.append(m)
    flags = dict(use_b2c=use_b2c, use_gamma=use_gamma, use_beta=use_beta)
    return in_maps, flags


def run(x, W1, b1, W2, b2, gamma, beta, trace=False, variant="full", **kw):
    from concourse.bass_utils import run_bass_kernel_spmd

    kw.pop("mm_f32r", None)
    in_maps, flags = prep_inputs(x, W1, b1, W2, b2, gamma, beta)
    nc = build_kernel(variant=variant, **flags)
    res = run_bass_kernel_spmd(
        nc, in_maps, core_ids=list(range(N_CORES)), trace=trace, **kw
    )
    y = np.concatenate([res.results[k]["y"] for k in range(N_CORES)], axis=0)
    return y.reshape(B, T, OUT).astype(np.float32), res


def kernel(x, W1, b1, W2, b2, gamma, beta):
    y, _ = run(x, W1, b1, W2, b2, gamma, beta)
    return y



# revision 58
# speedup vs baseline: 1.1362x; 1.0828x over previous
"""Trainium2 Bass kernel for nn_BottleneckFFN.

Computes y = LayerNorm(GELU(x @ W1.T + b1) @ W2.T + b2) * gamma + beta
for x of shape (128, 2048, 256), W1 (8, 256), W2 (8, 8), LN over the
trailing 8 channels.  Pure data parallel over 8 NeuronCores: the
128*2048 = 262144 token rows are split into 8 shards of 32768 tokens;
the tiny weights are replicated.

Per-core dataflow (per round of 2048 tokens), software-pipelined with a
1-round skew (loads/transposes for round r+1 are emitted before round
r's matmul stages):
  1. SWDGE cast-DMA (nc.gpsimd.dma_start with f32 in / bf16 out): the
     SDMA engines downcast inline, so 2 MB of HBM reads land directly
     as a 1 MB token-major bf16 tile ([128 part, 16 tiles, 256]).
     This deletes the old ACT(3.5us)+GpSimd(1.9us) per-round cast
     stage entirely -- the v1 trace showed those casts (plus the DVE
     transpose) starving the PE into HAM-cold oscillation (45%
     throttle time) and stretching the post-DMA drain to ~50us.
     bf16 matmuls keep full PE speed without float32r's
     psum-partition-base-0 restriction.  Measured rel err 3.5e-3
     (gate 2e-2).  Descriptor generation (~1us/round) runs on the
     GpSimd engine, which has nothing else to do.
  2. Two DVE 32x32 block transposes to feature-major per 32-partition
     group, on the bf16 tile BITCAST TO i32 so each transposed element
     is a packed pair of adjacent features: DVE transpose is
     element-rate-limited (~1.05 ns/col regardless of width), so the
     i32 packing halves DVE transpose time.  mm1 then reads even/odd
     features as stride-2 bf16 APs (per-column partition-parallel
     fetch makes PE cost stride-independent), with W1 reordered
     host-side to match (K-step k = 2*db32 + e contracts
     d = 64*db32 + 2a + e at partition 32P+a).
  3. mm1: 8 d-blocks x 4 concurrent diagonal K=32 bf16 matmuls
     (tile_position (32P, 32P)) accumulate x @ W1.T into ONE psum bank
     as [128, 512]: token group P's channels land at partitions
     32P..32P+32 (same-bank different-partition writes are safe).
     pp bufs=3 (and pp2 bufs=3; 6 of 8 banks) buffers the bank so
     mm1(r+1..r+2) overlap GELU(r).
  4. Exact GELU over all 128 lanes, b1 fused as per-partition bias,
     bf16 output feeding mm2 directly.  With the cast gone, GELU
     (~0.6us) is ACT's only steady-state work.
  5. mm2: 4 concurrent diagonal K=8 bf16 matmuls with a 32-col
     stationary whose col 8 is mean(W2 rows), so the per-token LN mean
     falls out of the matmul; fresh double-buffered psum bank.
  6. One DVE block-transpose back to token-major; centered = h2 - mu,
     Square, grouped reduce, all on DVE (~4.3us/round total DVE,
     under the 5.2us DMA round).
  7. Finalize per batch (rounds 0-8 / 8-14 / 14-16; the last batch's
     finalize is EMITTED after round 15's transpose+stats, so nothing
     queues ahead of the tail DVE chain):
     rstd = rsqrt(ssq/8 + eps) via magic-constant seed + one DVE
     Newton step -- table-free, so the ACT Gelu table is NEVER
     swapped (the old ACT Sqrt cost 2 x ~2.7us Gelu<->Sqrt table
     loads per finalize, parked between two GELUs that mm2 was
     waiting on).  The whole batch is then scaled in ONE wide DVE
     tensor_tensor (cent * rstd-broadcast over nr*128 cols) and
     stored with ONE multi-round HWDGE dma_start through a p-outer
     view of y (per partition the batch is a (round, f) 2D walk), so
     per-round scale/store op overhead and the DVE tail backlog both
     collapse; stores go out on the idle SP engine so nothing shares
     the GpSimd SWDGE descgen or the ACT GELU stream.

Key scheduling facts learned from traces (see git-less history in
kernel_v*.py):
  - The PE queue is in-order: mm2(r) emitted right after mm1(r) made
    the PE idle ~1.2-1.5us/round waiting on GELU(r).  Emitting mm2 one
    round late (after mm1(r+1)) removed the bubble: 126.8 -> 113.7us.
  - Round 0 is loaded as 4 independent sub-tiles with per-sub
    transposes and j-split mm1 so first matmuls issue at ~12.5us
    instead of ~15.5 (shorter fill, earlier HAM warm-up).
  - Steady state is at the HBM read roofline (~4.9-5.2us/round,
    ~430 GB/s effective); DVE is the closest compute engine
    (~4.6us/round).  Moving stats/scales to GpSimd (2-input TT) or
    folding the LN-mean into a centered mm2 stationary with persistent
    yt tiles both REGRESSED >13us on HW (GpSimd elementwise is far
    slower than its 2x-of-DVE billing; the persistent-yt variant
    re-cooled the PE) -- both were reverted.

v1 (HWDGE f32 loads + on-engine casts) measured 136.8us: loads ran at
~425 GB/s and finished by t=88us, but the cast+transpose+stats chain
could only retire rounds at ~6-10us in the drain, and the HAM-cold PE
(545ns/matmul = 1.2 GHz) stretched mm1.  Removing the cast (v2,
126.8us), the table-free finalize (v3), the mm2 bubble fix (v4,
113.7us), the round-0 sub-split (v5, 112.9us) and the merged
batch-wide scale+store (v8, 112.4us) landed at ~6.5us preamble +
~6us fill + ~80us of DMA-roofline rounds + ~13us tail+barrier.
Beware ~13% run-to-run drift on this part (HAM clock-gate phase +
cross-core HBM contention): identical binaries measured 112.9 and
128.7 in one session.  PE "heater" tricks were tried twice and
reverted: a 3.7us burst of K=32 matmuls at t=7.7-11.5 (memset-fed, so
it really did run in the fill window) did NOT trip HAM's un-throttle
-- the first K=8/8 event still came at t=33us -- so the activity
monitor evidently discounts low-array-utilization matmuls (1-4 of 16
sub-arrays; even the real diagonal mm1 only lights 4).  The warm/cold
lottery cannot be steered from the instruction stream; don't burn PE
time trying.
"""

import os
import sys

import numpy as np

if not any(os.path.isdir(os.path.join(p, "concourse")) for p in sys.path if p):
    for _cand in ("/opt/trn_rl_repo", "/root/.axon_site/_ro/trn_rl_repo"):
        if os.path.isdir(os.path.join(_cand, "concourse")):
            sys.path.insert(0, _cand)
            break

N_CORES = 8
DIM, OUT = 256, 8
B, T = 128, 2048
TOK_TOTAL = B * T
TOK_CORE = TOK_TOTAL // N_CORES  # 32768
R_TOK = 2048                     # tokens per round
N_R = TOK_CORE // R_TOK          # 16 rounds
J = R_TOK // 128                 # 16 [128, 256] tiles per round
JH = J // 2                      # 8 tiles per half-round
NDB = DIM // 32                  # 8 d-blocks of 32
EPS = 1e-5

_BUILD_CACHE = {}

# CoreSim doesn't implement Gelu; sim_test.py swaps in Tanh (and mirrors
# it in its numpy reference) to validate dataflow/layout off-hardware.
SIM_ACT_OVERRIDE = [None]


def build_kernel(use_b2c=False, use_gamma=False, use_beta=False,
                 repeat=1, variant="full"):
    """Build the per-core Bass program. Returns the compiled Bacc object."""
    key = (use_b2c, use_gamma, use_beta, repeat, variant,
           str(SIM_ACT_OVERRIDE[0]))
    if key in _BUILD_CACHE:
        return _BUILD_CACHE[key]

    import concourse.bacc as bacc
    import concourse.mybir as mybir
    from concourse.tile import TileContext

    f32 = mybir.dt.float32
    bf16 = mybir.dt.bfloat16
    AF = mybir.ActivationFunctionType
    ALU = mybir.AluOpType

    nc = bacc.Bacc("TRN2")
    x_d = nc.dram_tensor("x", [TOK_CORE, DIM], f32, kind="ExternalInput")
    # f32 consts: col 0 b1 (replicated per 32-group), 8:16 b2-mean(b2),
    # 16:24 gamma, 24:32 beta
    wp_d = nc.dram_tensor("wpack", [128, 32], f32, kind="ExternalInput")
    # bf16 consts: cols 0:256 w1t blocks, 256:288 w2t9 (replicated per
    # 32-group)
    wb_d = nc.dram_tensor("wpackb", [128, 288], bf16, kind="ExternalInput")
    y_d = nc.dram_tensor("y", [TOK_CORE, OUT], f32, kind="ExternalOutput")

    # token t = r*2048 + p*16 + f: each partition reads one contiguous
    # 16 KB run per round and writes one contiguous 512 B run.
    x_v = x_d[:, :].rearrange("(r p f) d -> r p f d", r=N_R, p=128, f=J)
    y_v = y_d[:, :].rearrange("(r p f) c -> r p f c", r=N_R, p=128, f=J)
    # p-OUTER view of y: y_vp[:, r0:r1] is a single multi-round store
    # AP ([128, k rounds, J, 8] -- per partition a (r, f) 2D walk), so
    # one finalize batch stores with ONE HWDGE dma_start.
    y_vp = y_d[:, :].rearrange("(r p f) c -> p r f c", r=N_R, p=128, f=J)

    with TileContext(nc) as tc:
        with (
            tc.tile_pool(name="consts", bufs=1) as consts,
            tc.tile_pool(name="x0p", bufs=1) as x0p,
            tc.tile_pool(name="xcp", bufs=8) as xcp,
            tc.tile_pool(name="xtp", bufs=3) as xtp,
            tc.tile_pool(name="h1p", bufs=3) as h1p,
            tc.tile_pool(name="ytp", bufs=3) as ytp,
            tc.tile_pool(name="sqp", bufs=2) as sqp,
            tc.tile_pool(name="accp", bufs=1) as accp,
            tc.tile_pool(name="yout", bufs=1) as yout,
            tc.tile_pool(name="pp", bufs=3, space="PSUM") as pp,
            tc.tile_pool(name="pp2", bufs=3, space="PSUM") as pp2,
        ):
            wp = consts.tile([128, 32], f32)
            nc.sync.dma_start(out=wp, in_=wp_d[:, :])
            wb = consts.tile([128, 288], bf16)
            nc.sync.dma_start(out=wb, in_=wb_d[:, :])
            w1t = wb[:, 0:DIM]
            w2t = wb[:, DIM : DIM + 32]
            b1c = wp[:, 0:1]
            aux = wp[:, 8:32]

            # finalize batches: the last batch's finalize is EMITTED
            # after round 15's transpose+stats (see one_pass), so the
            # tail DVE stream is [yt15, sq15, red15, one rsqrt chain,
            # one scale, one store] with nothing queued ahead of it.
            BATCHES = [(0, 8), (8, 14), (14, 16)]

            # split accumulators per finalize batch: no shared tile
            # between in-flight rounds and a draining finalize.
            cent_b = [
                accp.tile([128, (hi - lo) * 128], f32, name=f"cent{b}",
                          tag=f"cent{b}")
                for b, (lo, hi) in enumerate(BATCHES)
            ]
            ssq_b = [
                accp.tile([128, (hi - lo) * 16], f32, name=f"ssq{b}",
                          tag=f"ssq{b}")
                for b, (lo, hi) in enumerate(BATCHES)
            ]

            def batch_of(r):
                for b, (lo, hi) in enumerate(BATCHES):
                    if lo <= r < hi:
                        return b, r - lo
                raise AssertionError(r)

            def dma_only_pass():
                for r in range(N_R):
                    xc = xcp.tile([128, J, DIM], bf16, tag="xc")
                    nc.gpsimd.dma_start(out=xc, in_=x_v[r])
                    y_t = yout.tile([128, J, 8], f32, tag="y_t")
                    nc.vector.tensor_copy(out=y_t[:, 0:1, :], in_=xc[:, 0:1, 0:8])
                    nc.sync.dma_start(out=y_v[r], in_=y_t)

            rstd_b = [
                accp.tile([128, (hi - lo) * 16], f32, name=f"rstd{b}",
                          tag=f"rstd{b}")
                for b, (lo, hi) in enumerate(BATCHES)
            ]

            def finalize(b):
                # rstd = rsqrt(ssq/8 + eps) via magic-constant seed
                # (int ops on GpSimd) + one DVE Newton step (~0.2% max
                # err, in the noise next to bf16's 3.5e-3): no ACT
                # involvement, so the Gelu table is NEVER switched out
                # (the old ACT Sqrt cost 2 x ~2.7us Gelu<->Sqrt table
                # loads per finalize, parked in the ACT stream right
                # between two GELUs that mm2 was waiting on).
                r_lo, r_hi = BATCHES[b]
                nr = r_hi - r_lo
                n = nr * 16
                i32 = mybir.dt.int32
                v = sqp.tile([128, n], f32, tag="vv")
                nc.vector.tensor_scalar(
                    out=v, in0=ssq_b[b], scalar1=1.0 / OUT, scalar2=EPS,
                    op0=ALU.mult, op1=ALU.add,
                )
                # y0 bits = 0x5f3759df - (v_bits >> 1)
                #         = ((v_bits >> 1) ^ -1) + 0x5f3759e0
                # (on DVE: walrus rejects TensorScalar on Pool/GpSimd)
                t = sqp.tile([128, n], i32, tag="t0")
                nc.vector.tensor_scalar(
                    out=t, in0=v.bitcast(i32), scalar1=1, scalar2=-1,
                    op0=ALU.logical_shift_right, op1=ALU.bitwise_xor,
                )
                nc.vector.tensor_scalar(
                    out=t, in0=t, scalar1=0x5F3759E0, scalar2=None,
                    op0=ALU.add,
                )
                y0 = t.bitcast(f32)
                a = sqp.tile([128, n], f32, tag="na")
                nc.vector.tensor_tensor(out=a, in0=y0, in1=y0, op=ALU.mult)
                nc.vector.tensor_tensor(out=a, in0=v, in1=a, op=ALU.mult)
                nc.vector.tensor_scalar(
                    out=a, in0=a, scalar1=-0.5, scalar2=1.5,
                    op0=ALU.mult, op1=ALU.add,
                )
                nc.vector.tensor_tensor(
                    out=rstd_b[b], in0=y0, in1=a, op=ALU.mult
                )
                # ---- merged scale + ONE multi-round store for the
                # whole batch: one DVE TT (nr*128 cols) replaces nr
                # separate 128-col TTs (saves the per-op fixed cost and
                # shrinks the DVE tail backlog), and the p-outer y view
                # makes the nr-round store a single HWDGE dma_start ----
                m = nr * 16
                y_t = yout.tile([128, nr, J, 8], f32, tag=f"y_t{b}")
                y_tv = y_t.rearrange("p r j c -> p (r j) c")
                cent_v = cent_b[b].rearrange("p (m c) -> p m c", c=8)
                rs = rstd_b[b].rearrange("p (m c) -> p m c", c=1)
                rs = rs.broadcast_to([128, m, 8])
                nc.vector.tensor_tensor(
                    out=y_tv, in0=cent_v, in1=rs, op=ALU.mult
                )
                if use_gamma:
                    gm = aux[:, 8:16].rearrange(
                        "p (j c) -> p j c", j=1
                    ).broadcast_to([128, m, 8])
                    nc.vector.tensor_tensor(
                        out=y_tv, in0=y_tv, in1=gm, op=ALU.mult
                    )
                if use_beta:
                    bt = aux[:, 16:24].rearrange(
                        "p (j c) -> p j c", j=1
                    ).broadcast_to([128, m, 8])
                    nc.vector.tensor_tensor(
                        out=y_tv, in0=y_tv, in1=bt, op=ALU.add
                    )
                nc.sync.dma_start(out=y_vp[:, r_lo:r_hi], in_=y_t)

            def load_x(r):
                # ---- SWDGE cast-DMA: f32 HBM rows -> bf16 SBUF,
                # token-major; the SDMA engines downcast inline ----
                xc = xcp.tile([128, J, DIM], bf16, tag="xc")
                nc.gpsimd.dma_start(out=xc, in_=x_v[r])
                return xc

            def transpose_x(xc):
                # ---- 32x32 block transpose to feature-major, on PACKED
                # u32 pairs: DVE transpose is element-rate-limited
                # (~1.05ns/col regardless of width), so transposing
                # bf16 pairs as one i32 element halves DVE time.
                # u32 col c32 = 32*db32 + b holds features (2c32, 2c32+1),
                # so xt[32P+a, j, 64*db32 + 2b + e] (bf16 view)
                #   = x[token r*2048 + j*128 + 32P + b, d = 64*db32+2a+e]
                # and the PE reads each (db32, e) slice as a stride-2 AP
                # (per-column partition-parallel fetch; stride-free cost).
                # TWO [128, 8, 128] ops, NOT one [128, 16, 128]: the
                # merged op was tried and measured 2312ns vs 2x1042ns
                # -- the half-round shape is the faster DVE mode, the
                # per-op overhead is negative.  Keep the split.
                i32 = mybir.dt.int32
                xt = xtp.tile([128, J, DIM // 2], i32, tag="xt")
                xci = xc.bitcast(i32)
                for q in range(2):
                    w = J // 2
                    nc.vector.transpose(
                        out=xt[:, w * q : w * (q + 1), :],
                        in_=xci[:, w * q : w * (q + 1), :],
                    )
                return xt

            yts = {}

            def stats(r):
                b, i = batch_of(r)
                yt = yts.pop(r)
                cent = cent_b[b][:, i * 128 : (i + 1) * 128].rearrange(
                    "p (j c) -> p j c", c=8
                )
                # whole yt->cent->sq->reduce chain on DVE: it has ~2us
                # of slack since the packed transpose, and keeping the
                # chain single-engine removes two cross-engine hops of
                # latency per round (dominant in the post-DMA drain)
                mu = yt[:, :, 8:9].broadcast_to([128, J, 8])
                nc.vector.tensor_tensor(
                    out=cent, in0=yt[:, :, 0:8], in1=mu, op=ALU.subtract
                )
                if use_b2c:
                    b2c = aux[:, 0:8].rearrange(
                        "p (j c) -> p j c", j=1
                    ).broadcast_to([128, J, 8])
                    nc.vector.tensor_tensor(
                        out=cent, in0=cent, in1=b2c, op=ALU.add
                    )
                sq = sqp.tile([128, 128], f32, tag="sq")
                nc.vector.tensor_tensor(
                    out=sq,
                    in0=cent_b[b][:, i * 128 : (i + 1) * 128],
                    in1=cent_b[b][:, i * 128 : (i + 1) * 128],
                    op=ALU.mult,
                )
                nc.vector.reduce_sum(
                    out=ssq_b[b][:, i * 16 : (i + 1) * 16],
                    in_=sq.rearrange("p (j c) -> p j c", c=8),
                    axis=mybir.AxisListType.X,
                )

            def mm1_round(r, xts, splits=1):
                # bf16 view of the u32-transposed tile:
                # col (db32, b, e) = 64*db32 + 2b + e
                xt_b = xts.pop(r).bitcast(bf16).rearrange(
                    "p j (db b e) -> p j db e b", db=4, b=32, e=2
                )
                # ---- mm1: 4 diagonal streams, one full psum bank;
                # 8 K-steps k = 2*db32 + e over the packed layout.
                # splits>1 (round 0 only) chops the round into j-column
                # groups so the first matmuls issue as soon as the first
                # sub-load's transpose lands, shortening the pipeline
                # fill and warming the PE's HAM clock early ----
                ps = pp.tile([128, 512], f32, name="ps", tag="ps")
                jw = J // splits
                for s in range(splits):
                    for db in range(NDB):
                        db32, e = divmod(db, 2)
                        for P in range(4):
                            nc.tensor.matmul(
                                out=ps[
                                    32 * P : 32 * P + 32,
                                    s * jw * 32 : (s + 1) * jw * 32,
                                ],
                                lhsT=w1t[
                                    32 * P : 32 * P + 32,
                                    32 * db : 32 * db + 32,
                                ],
                                rhs=xt_b[
                                    32 * P : 32 * P + 32,
                                    s * jw : (s + 1) * jw,
                                    db32, e, :,
                                ],
                                start=(db == 0),
                                stop=(db == NDB - 1),
                                tile_position=(32 * P, 32 * P),
                                skip_group_check=True,
                            )
                # ---- exact GELU (erf) on all 128 lanes, + b1,
                # bf16 out feeding mm2; runs on ACT while the PE is
                # already into mm1(r+1) ----
                h1 = h1p.tile([128, 512], bf16, tag="h1")
                nc.scalar.activation(
                    out=h1, in_=ps,
                    func=SIM_ACT_OVERRIDE[0] or AF.Gelu,
                    bias=b1c, scale=1.0,
                )
                return h1

            def mm2_stats_round(r, h1):
                # ---- mm2: 4 diagonal K=8 streams.  EMITTED AFTER
                # mm1(r+1): the PE queue is in-order, so putting mm2(r)
                # right after mm1(r) made the PE sit idle ~1.2-1.5us
                # every round waiting for GELU(r).  One round of skew
                # lets GELU(r) finish while the PE runs mm1(r+1), so
                # mm2(r) issues bubble-free ----
                ps2 = pp2.tile([128, 512], f32, name="ps2", tag="ps2")
                for g in range(4):
                    nc.tensor.matmul(
                        out=ps2[32 * g : 32 * g + 32, :],
                        lhsT=w2t[32 * g : 32 * g + 8, 0:32],
                        rhs=h1[32 * g : 32 * g + 8, :],
                        start=True,
                        stop=True,
                        tile_position=(32 * g, 32 * g),
                        skip_group_check=True,
                    )
                yt = ytp.tile([128, J, 32], f32, tag="yt")
                # ONE op, not two halves: splitting was tried (the xt
                # transposes measured faster as halves) but yt reads
                # PSUM, and each op pays the ~120-cycle PSUM access
                # latency -- halves measured 2 x ~410ns vs 674ns.
                nc.vector.transpose(out=yt, in_=ps2[:, :])
                # yt[p, j, c]: c 0..7 = h2 channels, c 8 = mean
                yts[r] = yt
                stats(r)
                # the LAST batch's finalize is emitted by one_pass
                # after round 15's stats, so round-15's yt/stats ops
                # aren't queued on DVE behind an earlier batch's
                # rsqrt/scale chain in the kernel tail
                for fb, (lo, hi) in enumerate(BATCHES):
                    if r == hi - 1 and hi < N_R:
                        finalize(fb)

            def one_pass():
              if variant == "dmaonly":
                  dma_only_pass()
                  return
              # software-pipelined: loads and transposes for round r+1
              # are EMITTED before round r's mm1, and mm2/stats run one
              # round behind mm1 (see mm2_stats_round).
              xts = {}
              h1s = {}
              # round 0 arrives as 4 independent sub-loads, each with
              # its own tile so its transpose waits only on its own
              # bytes -- first mm1 matmuls issue ~4us sooner.
              i32 = mybir.dt.int32
              SUBJ = J // 4
              x0s = []
              for s in range(4):
                  x0 = x0p.tile([128, SUBJ, DIM], bf16, tag=f"x0_{s}")
                  nc.gpsimd.dma_start(
                      out=x0, in_=x_v[0][:, s * SUBJ : (s + 1) * SUBJ, :]
                  )
                  x0s.append(x0)
              xcs = {q: load_x(q) for q in range(1, 5)}
              xt0 = xtp.tile([128, J, DIM // 2], i32, tag="xt")
              for s in range(4):
                  nc.vector.transpose(
                      out=xt0[:, s * SUBJ : (s + 1) * SUBJ, :],
                      in_=x0s[s].bitcast(i32),
                  )
              xts[0] = xt0
              for r in range(N_R):
                  if r + 5 < N_R:
                      xcs[r + 5] = load_x(r + 5)
                  if r + 1 < N_R:
                      xts[r + 1] = transpose_x(xcs.pop(r + 1))
                  h1s[r] = mm1_round(r, xts, splits=(4 if r == 0 else 1))
                  if r >= 1:
                      mm2_stats_round(r - 1, h1s.pop(r - 1))
              mm2_stats_round(N_R - 1, h1s.pop(N_R - 1))
              finalize(len(BATCHES) - 1)

            for _rep in range(repeat):
                one_pass()

    nc.compile()
    _BUILD_CACHE[key] = nc
    return nc


def prep_inputs(x, W1, b1, W2, b2, gamma, beta):
    """Host-side prep: shard x, lay out the tiny weights for the kernel."""
    import ml_dtypes

    x = np.ascontiguousarray(np.asarray(x, dtype=np.float32)).reshape(TOK_TOTAL, DIM)
    W1 = np.asarray(W1, dtype=np.float32)
    b1 = np.asarray(b1, dtype=np.float32)
    W2 = np.asarray(W2, dtype=np.float32)
    b2 = np.asarray(b2, dtype=np.float32)
    gamma = np.asarray(gamma, dtype=np.float32)
    beta = np.asarray(beta, dtype=np.float32)

    # packed-pair layout: K-step k = 2*db32 + e contracts features
    # d = 64*db32 + 2a + e at partition 32P+a, so
    # w1t[32P+a, 32k+o] = W1[o, 64*(k//2) + 2a + (k%2)], replicated per P
    kk = np.arange(NDB)
    aa = np.arange(32)
    dmat = 64 * (kk[:, None] // 2) + 2 * aa[None, :] + (kk[:, None] % 2)
    w1g = np.zeros((32, NDB, 32), np.float32)            # [a, k, oslot]
    w1g[:, :, :OUT] = W1[:, dmat].transpose(2, 1, 0)     # [o,k,a]->[a,k,o]
    w1t = np.tile(w1g.reshape(32, DIM), (4, 1))

    # w2t9[32g+o, m] = W2[m, o] (o < 8); col 8 = mean over rows of W2,
    # replicated into each 32-partition group
    w2t9 = np.zeros((32, 32), np.float32)
    w2t9[:OUT, :OUT] = W2.T
    w2t9[:OUT, 8] = W2.mean(axis=0)
    w2rep = np.tile(w2t9, (4, 1))

    use_b2c = bool(np.any(b2 != 0.0))
    use_gamma = bool(np.any(gamma != 1.0))
    use_beta = bool(np.any(beta != 0.0))

    wpackb = np.zeros((128, 288), ml_dtypes.bfloat16)
    wpackb[:, 0:DIM] = w1t.astype(ml_dtypes.bfloat16)
    wpackb[:, DIM : DIM + 32] = w2rep.astype(ml_dtypes.bfloat16)

    wpack = np.zeros((128, 32), np.float32)
    b1full = np.zeros((128,), np.float32)
    for g in range(4):
        b1full[32 * g : 32 * g + OUT] = b1
    wpack[:, 0] = b1full
    wpack[:, 8:16] = (b2 - b2.mean())[None, :]
    wpack[:, 16:24] = gamma[None, :]
    wpack[:, 24:32] = beta[None, :]

    in_maps = []
    for k in range(N_CORES):
        m = {
            "x": np.ascontiguousarray(x[k * TOK_CORE : (k + 1) * TOK_CORE]),
            "wpack": wpack,
            "wpackb": wpackb,
        }
        in_maps.append(m)
    flags = dict(use_b2c=use_b2c, use_gamma=use_gamma, use_beta=use_beta)
    return in_maps, flags


def run(x, W1, b1, W2, b2, gamma, beta, trace=False, variant="full", **kw):
    from concourse.bass_utils import run_bass_kernel_spmd

    kw.pop("mm_f32r", None)
    in_maps, flags = prep_inputs(x, W1, b1, W2, b2, gamma, beta)
    nc = build_kernel(variant=variant, **flags)
    res = run_bass_kernel_spmd(
        nc, in_maps, core_ids=list(range(N_CORES)), trace=trace, **kw
    )
    y = np.concatenate([res.results[k]["y"] for k in range(N_CORES)], axis=0)
    return y.reshape(B, T, OUT).astype(np.float32), res


def kernel(x, W1, b1, W2, b2, gamma, beta):
    y, _ = run(x, W1, b1, W2, b2, gamma, beta)
    return y

